# revision 23
# baseline (speedup 1.0000x reference)
"""Causal self-attention (B=4, T=2048, C=768, H=12, RoPE) on 8 TRN2 NeuronCores.

Sharding: core c -> (batch b = c//2, head-group g = c%2 of 6 heads).
Each core computes, for its batch element and its 6 heads:
    qkv projections, RoPE, causal attention, and the partial output
    projection  attn_out @ W_proj[rows of its heads].
Host sums the two partial outputs per batch and adds the (host-folded)
bias terms:  out[b] = part[2b] + part[2b+1] + b_proj + b_v @ W_proj.

v3 design (software-pipelined, bf16, DMA-lean):
  - All matmul operands bf16 (inputs pre-cast on host); psum f32.
  - Few, large input DMAs (HWDGE descriptor-gen serializes ~630ns/DMA).
  - RoPE row-swap done with a PE permutation matmul (no SBUF-SBUF DMAs).
  - Emission interleaving: pair-0 QK/V chunks are staged right before the
    query block that needs them; QK of pair j+1 and the output projection
    run as credit-scheduled "filler" PE work inside the ACT(exp)-bound
    attention loop, so no engine idles between phases.
  - Exact-causal trimming: the 4 diagonal key-chunks of each 512-query
    block run at widths 512/384/256/128; only the boundary [128,128]
    triangle gets the affine_select mask.
  - Scores computed transposed: S^T[k, q] = K^T-chunk @ Q-chunk, exp on
    ACT (scale=1/8), then Y'^T = [V|1]^T-chain accumulation giving Y^T
    (rows 0-63) and softmax denominators (row 64).  Normalization via
    DVE reciprocal + DRAM-broadcast multiply.
"""
import sys
sys.path.insert(0, "/opt/trn_rl_repo")

import numpy as np
import ml_dtypes

ROPE_BASE = 10000.0
NCORES = 8
BF16NP = ml_dtypes.bfloat16

_CACHE = {}


def _rope_tables(T):
    inv_freq = 1.0 / (ROPE_BASE ** (np.arange(0, 64, 2, dtype=np.float64) / 64))
    t = np.arange(T, dtype=np.float64)
    fr = np.outer(t, inv_freq)            # [T, 32]
    cosT = np.cos(fr).T                   # [32, T]
    sinT = np.sin(fr).T
    CC = np.tile(cosT, (4, 1)).astype(BF16NP)                      # [128, T]
    SS = np.concatenate([sinT, -sinT, sinT, -sinT], 0).astype(BF16NP)
    return CC, SS


def _swap_perm():
    """[128,128] P with P[swap(p), p] = 1 so (P.T @ x)[p] = x[swap(p)],
    swap = exchange rows 0-31<->32-63 and 64-95<->96-127."""
    P = np.zeros((128, 128), dtype=BF16NP)
    for p in range(128):
        h, r = divmod(p, 64)
        P[h * 64 + (r + 32) % 64, p] = 1.0
    return P


def build_nc(C, T, HPC, loop_n=1):
    """Per-core Bass program. C: contraction dim, T: seq len, HPC: heads."""
    import concourse.bass as bass
    import concourse.tile as tile
    from concourse import bacc, mybir
    import contextlib
    from collections import deque

    F32 = mybir.dt.float32
    BF16 = mybir.dt.bfloat16
    Act = mybir.ActivationFunctionType

    KT = C // 128          # contraction k-tiles (6)
    NP = HPC // 2          # head pairs (3)
    TT = T // 128          # 128-row t-tiles / key chunks (16)
    QC = T // 512          # query blocks of 512 (4)
    VC = 64 * HPC          # v columns per core (384)
    PC = VC // 128         # projection contraction k-tiles (3)

    nc = bacc.Bacc("TRN2", target_bir_lowering=False, debug=False)

    # wqk is host-packed pair-major: [C, NP, 256] = [.., j, (q_j|k_j)]
    xt_d = nc.dram_tensor("xt", [C, T], BF16, kind="ExternalInput")
    wqk_d = nc.dram_tensor("wqk", [C, 2 * VC], BF16, kind="ExternalInput")
    bqk_d = nc.dram_tensor("bqk", [2 * VC], F32, kind="ExternalInput")
    wv_d = nc.dram_tensor("wv", [C, VC], BF16, kind="ExternalInput")
    wp_d = nc.dram_tensor("wp", [VC, C], BF16, kind="ExternalInput")
    out_d = nc.dram_tensor("out", [T, C], F32, kind="ExternalOutput")

    CCh, SSh = _rope_tables(T)
    cc_d = nc.inline_tensor(CCh, name="rope_cc")
    ss_d = nc.inline_tensor(SSh, name="rope_ss")
    psw_d = nc.inline_tensor(_swap_perm(), name="rope_swap")

    @contextlib.contextmanager
    def _maybe_loop(tc):
        if loop_n > 1:
            with tc.For_i(0, loop_n, 1):
                yield
        else:
            yield

    with nc.allow_low_precision(reason="bf16 matmul pipeline"), \
         tile.TileContext(nc) as tc:
        with contextlib.ExitStack() as octx, _maybe_loop(tc), \
             contextlib.ExitStack() as ctx:
            P = lambda name, bufs=1: ctx.enter_context(
                tc.tile_pool(name=name, bufs=bufs))
            sb = P("sb")
            xt_sb = sb.tile([128, KT, T], BF16, name="xt_sb")
            yt_sb = sb.tile([128, NP, T], BF16, name="yt_sb")
            wqk_sb = sb.tile([128, KT, NP, 256], BF16, name="wqk_sb")
            wv_sb = sb.tile([128, KT, VC], BF16, name="wv_sb")
            wp_sb = sb.tile([128, PC, C], BF16, name="wp_sb")
            # wp rows of the last pair's head-b, relocated to partitions 0-63
            wpb_sb = sb.tile([64, C], BF16, name="wpb_sb")
            # last pair's head-b Y rows (kept in SBUF; no partition-move DMA)
            tbs_sb = sb.tile([64, QC, 512], BF16, name="tbs_sb")
            vp_sb = sb.tile([128, TT, HPC, 65], BF16, name="vp_sb")
            cc_sb = sb.tile([128, T], BF16, name="cc_sb")
            ss_sb = sb.tile([128, T], BF16, name="ss_sb")
            psw_sb = sb.tile([128, 128], BF16, name="psw_sb")
            bqk_sb = sb.tile([128, 2 * VC // 128], F32, name="bqk_sb")

            qk_pool = P("qks", bufs=2)
            pt_pool = P("pt", bufs=3)
            raw_pool = P("raw", bufs=2)
            t1_pool = P("t1", bufs=2)
            u_pool = P("u", bufs=2)
            yraw_pool = P("yraw", bufs=2)
            rd_pool = P("rd", bufs=2)
            tb_pool = P("tb", bufs=2)
            osb_pool = P("osb", bufs=3)

            mm_ps = ctx.enter_context(
                tc.tile_pool(name="mmps", bufs=2, space="PSUM"))
            qk_ps = ctx.enter_context(
                tc.tile_pool(name="qkps", bufs=2, space="PSUM"))
            y_ps = ctx.enter_context(
                tc.tile_pool(name="yps", bufs=1, space="PSUM"))

            # ---------------- loads (few, large, priority-ordered) -----
            nc.scalar.dma_start(
                xt_sb[:, :, 0:512],
                xt_d.ap()[:, 0:512].rearrange("(k p) t -> p k t", p=128))
            nc.sync.dma_start(
                wqk_sb[:, :, 0, :],
                wqk_d.ap()[:, 0:256].rearrange("(k p) c -> p k c", p=128))
            nc.sync.dma_start(
                wv_sb[:],
                wv_d.ap().rearrange("(k p) v -> p k v", p=128))
            nc.sync.dma_start(
                bqk_sb[:], bqk_d.ap().rearrange("(m p) -> p m", p=128))
            nc.sync.dma_start(psw_sb, psw_d.ap())
            nc.scalar.dma_start(cc_sb, cc_d.ap())
            nc.scalar.dma_start(ss_sb, ss_d.ap())
            nc.scalar.dma_start(
                xt_sb[:, :, 512:1024],
                xt_d.ap()[:, 512:1024].rearrange("(k p) t -> p k t", p=128))
            nc.scalar.dma_start(
                xt_sb[:, :, 1024:2048],
                xt_d.ap()[:, 1024:2048].rearrange("(k p) t -> p k t", p=128))
            for j2 in (1, 2):
                nc.sync.dma_start(
                    wqk_sb[:, :, j2, :],
                    wqk_d.ap()[:, 256 * j2:256 * (j2 + 1)].rearrange(
                        "(k p) c -> p k c", p=128))
            nc.sync.dma_start(
                wp_sb[:],
                wp_d.ap().rearrange("(k p) c -> p k c", p=128))
            nc.sync.dma_start(wpb_sb, wp_d.ap()[VC - 64:VC, :])
            nc.vector.tensor_copy(
                vp_sb[:, :, :, 64:65],
                nc.const_aps.tensor(1.0, (128, TT, HPC, 1)))
            ones_sb = sb.tile([65, 64], BF16, name="ones_sb")
            nc.vector.memset(ones_sb[64:65, :], 1.0)

            # ---------------- emission helpers -------------------------
            pair_tiles = {}

            def get_pair(j):
                if j not in pair_tiles:
                    qt = qk_pool.tile([128, T], BF16, tag="qt", name=f"qt{j}")
                    kt = qk_pool.tile([128, T], BF16, tag="kt", name=f"kt{j}")
                    pair_tiles[j] = (qt, kt)
                return pair_tiles[j]

            def emit_qk_a(j, qk, ch):
                """QK chunk phase A: projection matmuls + bias -> raw."""
                cs = slice(ch * 512, (ch + 1) * 512)
                psum = qk_ps.tile([128, 512], F32, tag="qk", name="qkpsum")
                for k in range(KT):
                    nc.tensor.matmul(
                        psum, wqk_sb[:, k, j, qk * 128:(qk + 1) * 128],
                        xt_sb[:, k, cs], start=(k == 0), stop=(k == KT - 1))
                raw = raw_pool.tile([128, 512], BF16, name="raw")
                nc.vector.tensor_scalar_add(raw, psum, bqk_sb[:, 2 * j + qk:
                                                              2 * j + qk + 1])
                return psum, raw

            def emit_qk_b(j, qk, ch, psum, raw):
                """QK chunk phase B: rowswap via PE perm + RoPE combine.

                dst = raw*CC + rowswap(raw)*SS; the swap matmul overwrites
                the phase-A psum slot (its matmuls are already consumed).
                """
                dst = get_pair(j)[qk]
                cs = slice(ch * 512, (ch + 1) * 512)
                nc.tensor.matmul(psum, psw_sb, raw, start=True, stop=True)
                t1 = t1_pool.tile([128, 512], BF16, name="t1")
                nc.gpsimd.tensor_mul(t1, raw, cc_sb[:, cs])
                u = u_pool.tile([128, 512], BF16, name="u")
                nc.vector.tensor_mul(u, psum, ss_sb[:, cs])
                nc.vector.tensor_add(dst[:, cs], t1, u)

            def emit_qk_pair_chunks(j, ch):
                """Both m-tiles (q,k) of column chunk ch, phase-interleaved."""
                a0 = emit_qk_a(j, 0, ch)
                a1 = emit_qk_a(j, 1, ch)
                emit_qk_b(j, 0, ch, *a0)
                emit_qk_b(j, 1, ch, *a1)

            def emit_vp(tt):
                """V projection for key-chunk tt -> vp_sb[:, tt]."""
                psum = qk_ps.tile([128, 512], F32, tag="qk", name="vpsum")
                for k in range(KT):
                    nc.tensor.matmul(
                        psum[:, 0:VC], xt_sb[:, k, tt * 128:(tt + 1) * 128],
                        wv_sb[:, k, :], start=(k == 0), stop=(k == KT - 1))
                nc.vector.tensor_copy(
                    vp_sb[:, tt, :, 0:64],
                    psum[:, 0:VC].rearrange("p (h d) -> p h d", h=HPC))

            def emit_att_kc(j, qc, kc, qt, kt, yab, nkc):
                off = max(0, (kc - 4 * qc) * 128)
                ks = slice(kc * 128, (kc + 1) * 128)
                qs = slice(qc * 512 + off, (qc + 1) * 512)
                spair = mm_ps.tile([128, 1024], F32, tag="mm", name="spair")
                nc.tensor.matmul(spair[:, off:512], kt[0:64, ks], qt[0:64, qs],
                                 start=True, stop=True)
                nc.tensor.matmul(spair[:, 512 + off:1024], kt[64:128, ks],
                                 qt[64:128, qs], start=True, stop=True)
                pp = pt_pool.tile([128, 1024], BF16, tag="pp", name="pp")
                sview = spair.rearrange("p (h q) -> p h q", h=2)[:, :, off:512]
                pview = pp.rearrange("p (h q) -> p h q", h=2)[:, :, off:512]
                nc.scalar.activation(pview, sview, Act.Exp, scale=0.125)
                if kc >= 4 * qc:  # diagonal: mask k > q -> 0 in the triangle
                    tri = pp.rearrange("p (h q) -> p h q",
                                       h=2)[:, :, off:off + 128]
                    nc.gpsimd.affine_select(
                        out=tri, in_=tri,
                        compare_op=mybir.AluOpType.is_ge, fill=0.0,
                        base=0, channel_multiplier=-1,
                        pattern=[[0, 2], [1, 128]])
                nc.tensor.matmul(yab[:, off:512], vp_sb[:, kc, 2 * j, :],
                                 pp[:, off:512], start=(kc == 0),
                                 stop=(kc == nkc - 1), skip_group_check=True)
                nc.tensor.matmul(yab[:, 512 + off:1024],
                                 vp_sb[:, kc, 2 * j + 1, :],
                                 pp[:, 512 + off:1024], start=(kc == 0),
                                 stop=(kc == nkc - 1), skip_group_check=True)

            def emit_norm_a(j, qc, yab):
                """Free the Y' psum: copy rows to SBUF, recip denominators."""
                rd = rd_pool.tile([65, 1024], BF16, name="rd")
                yr = yraw_pool.tile([64, 1024], BF16, name="yr")
                nc.vector.tensor_copy(yr, yab[0:64, :])
                nc.vector.reciprocal(rd[64:65, :], yab[64:65, :])
                return rd, yr

            def emit_norm_b(j, qc, rd, yr):
                """Broadcast 1/denom across partitions via a K=1 matmul,
                then scale Y^T rows into yt_sb (head-b of the last pair goes
                to tbs_sb; the projection consumes it directly)."""
                qs = slice(qc * 512, (qc + 1) * 512)
                bc0 = qk_ps.tile([128, 512], F32, tag="qk", name="bc0")
                nc.tensor.matmul(bc0[0:64, :], ones_sb[64:65, :],
                                 rd[64:65, 0:512], start=True, stop=True)
                bc1 = qk_ps.tile([128, 512], F32, tag="qk", name="bc1")
                nc.tensor.matmul(bc1[0:64, :], ones_sb[64:65, :],
                                 rd[64:65, 512:1024], start=True, stop=True)
                nc.vector.tensor_mul(yt_sb[0:64, j, qs], yr[:, 0:512],
                                     bc0[0:64, :])
                if j == NP - 1:
                    nc.vector.tensor_mul(tbs_sb[:, qc, :], yr[:, 512:1024],
                                         bc1[0:64, :])
                else:
                    tb = tb_pool.tile([64, 512], BF16, name="tb")
                    nc.vector.tensor_mul(tb, yr[:, 512:1024], bc1[0:64, :])
                    nc.sync.dma_start(yt_sb[64:128, j, qs], tb)

            def emit_proj(tt):
                ts = slice(tt * 128, (tt + 1) * 128)
                qc2, toff = divmod(tt, 4)
                osb = osb_pool.tile([128, C], F32, name="osb")
                for cc2 in range(2):
                    cs = slice(cc2 * 384, (cc2 + 1) * 384)
                    psum = qk_ps.tile([128, 512], F32, tag="qk", name="ppsum")
                    for k in range(PC - 1):
                        nc.tensor.matmul(
                            psum[:, 0:384], yt_sb[:, k, ts], wp_sb[:, k, cs],
                            start=(k == 0), stop=False)
                    # last pair: head-a from yt rows 0-63, head-b from tbs_sb
                    nc.tensor.matmul(
                        psum[:, 0:384], yt_sb[0:64, PC - 1, ts],
                        wp_sb[0:64, PC - 1, cs], start=False, stop=False,
                        skip_group_check=True)
                    nc.tensor.matmul(
                        psum[:, 0:384],
                        tbs_sb[:, qc2, toff * 128:(toff + 1) * 128],
                        wpb_sb[:, cs], start=False, stop=True,
                        skip_group_check=True)
                    nc.vector.tensor_copy(osb[:, cs], psum[:, 0:384])
                nc.scalar.dma_start(out_d.ap()[ts, :], osb)

            # ---------------- pipelined emission ------------------------
            emit_qk_pair_chunks(0, 0)
            for tt in range(4):
                emit_vp(tt)

            fillers = deque()
            CREDIT = {0: 330.0, 1: 330.0, 2: 500.0}
            RESERVE = {0: 2, 1: 2, 2: 0}
            COST = {"qka": 1280.0, "qkb": 250.0, "proj": 1060.0}

            def make_qk_fillers(jn, ch):
                state = {}
                def fa(q, c=ch, j_=jn):
                    state[q] = emit_qk_a(j_, q, c)
                def fb(q, c=ch, j_=jn):
                    emit_qk_b(j_, q, c, *state[q])
                return [("qka", (jn, ch), lambda: fa(0)),
                        ("qka", (jn, ch), lambda: fa(1)),
                        ("qkb", (jn, ch), lambda: fb(0)),
                        ("qkb", (jn, ch), lambda: fb(1))]

            def pump(j, credit_box, bump=None, reserve=None):
                credit_box[0] += CREDIT[j] if bump is None else bump
                rsv = RESERVE[j] if reserve is None else reserve
                while (len(fillers) > rsv
                       and credit_box[0] >= COST[fillers[0][0]]):
                    kind, key, fn = fillers.popleft()
                    credit_box[0] -= COST[kind]
                    fn()

            def drain_for(jt, qc):
                """Emit all queued QK work for pair jt, chunks <= qc."""
                while fillers:
                    kind, key, fn = fillers[0]
                    if kind.startswith("qk") and key[0] == jt and key[1] <= qc:
                        fillers.popleft()
                        fn()
                    else:
                        break

            for j in range(NP):
                qt, kt = get_pair(j)
                if j < NP - 1:
                    for ch in range(QC):
                        fillers.extend(make_qk_fillers(j + 1, ch))
                credit_box = [0.0]
                for qc in range(QC):
                    if j > 0:
                        drain_for(j, qc)
                    if j == 2 and qc >= 1:
                        for tt in range(4 * (qc - 1), 4 * qc):
                            fillers.append(
                                ("proj", None, (lambda t=tt: emit_proj(t))))
                    nkc = 4 * (qc + 1)
                    yab = y_ps.tile([65, 1024], F32, tag="yab", name="yab")
                    for kc in range(nkc):
                        emit_att_kc(j, qc, kc, qt, kt, yab, nkc)
                        pump(j, credit_box)
                    norm_st = emit_norm_a(j, qc, yab)
                    # cover the recip->broadcast latency with PE work
                    if j == 0 and qc < QC - 1:
                        a0 = emit_qk_a(0, 0, qc + 1)
                        a1 = emit_qk_a(0, 1, qc + 1)
                        emit_norm_b(j, qc, *norm_st)
                        emit_qk_b(0, 0, qc + 1, *a0)
                        emit_qk_b(0, 1, qc + 1, *a1)
                        for tt in range(4 * qc + 4, 4 * qc + 8):
                            emit_vp(tt)
                    else:
                        pump(j, credit_box, bump=1600.0, reserve=0)
                        emit_norm_b(j, qc, *norm_st)
            while fillers:
                kind, key, fn = fillers.popleft()
                fn()
            for tt in range(4 * (QC - 1), TT):
                emit_proj(tt)

    nc.compile()
    return nc


class _Runner:
    """Cached-jit SPMD runner (mirrors bass2jax.run_bass_via_pjrt, reusable)."""

    def __init__(self, nc, n_cores):
        import jax
        from jax.sharding import Mesh, PartitionSpec
        from jax.experimental.shard_map import shard_map
        import concourse.mybir as mybir
        from concourse import bass2jax

        bass2jax.install_neuronx_cc_hook()
        self.n_cores = n_cores
        part_name = (nc.partition_id_tensor.name
                     if nc.partition_id_tensor is not None else None)
        in_names, out_names, out_avals, zero_outs = [], [], [], []
        for alloc in nc.m.functions[0].allocations:
            if not isinstance(alloc, mybir.MemoryLocationSet):
                continue
            name = alloc.memorylocations[0].name
            if alloc.kind == "ExternalInput":
                if name != part_name:
                    in_names.append(name)
            elif alloc.kind == "ExternalOutput":
                out_names.append(name)
                shape = tuple(alloc.tensor_shape)
                dtype = mybir.dt.np(alloc.dtype)
                out_avals.append(jax.core.ShapedArray(shape, dtype))
                zero_outs.append(np.zeros(shape, dtype))
        self.in_names, self.out_names = in_names, out_names
        self.out_avals, self.zero_outs = out_avals, zero_outs
        all_names = in_names + out_names
        if part_name is not None:
            all_names = all_names + [part_name]

        def _body(*args):
            operands = list(args)
            if part_name is not None:
                operands.append(bass2jax.partition_id_tensor())
            return tuple(bass2jax._bass_exec_p.bind(
                *operands,
                out_avals=tuple(out_avals),
                in_names=tuple(all_names),
                out_names=tuple(out_names),
                lowering_input_output_aliases=(),
                sim_require_finite=True,
                sim_require_nnan=True,
                nc=nc,
            ))

        devices = jax.devices()[:n_cores]
        mesh = Mesh(np.asarray(devices), ("core",))
        nin = len(in_names) + len(out_names)
        self._fn = jax.jit(
            shard_map(_body, mesh=mesh,
                      in_specs=(PartitionSpec("core"),) * nin,
                      out_specs=(PartitionSpec("core"),) * len(out_names),
                      check_rep=False),
            keep_unused=True)

    def run(self, in_maps):
        args = [np.concatenate([np.asarray(m[name]) for m in in_maps], axis=0)
                for name in self.in_names]
        args += [np.zeros((self.n_cores * z.shape[0], *z.shape[1:]), z.dtype)
                 for z in self.zero_outs]
        outs = self._fn(*args)
        res = []
        for c in range(self.n_cores):
            d = {}
            for i, name in enumerate(self.out_names):
                per = np.asarray(outs[i]).reshape(
                    self.n_cores, *self.out_avals[i].shape)
                d[name] = per[c]
            res.append(d)
        return res


def _run(nc, in_maps):
    key = ("runner", id(nc))
    if key not in _CACHE:
        _CACHE[key] = _Runner(nc, len(in_maps))
    import types
    return types.SimpleNamespace(results=_CACHE[key].run(in_maps))


def make_in_maps(inputs):
    x = np.ascontiguousarray(np.asarray(inputs["x"], dtype=np.float32))
    W = np.asarray(inputs["W_attn"], dtype=np.float32)
    b = np.asarray(inputs["b_attn"], dtype=np.float32)
    Wp = np.asarray(inputs["W_proj"], dtype=np.float32)
    in_maps = []
    for c in range(NCORES):
        bb, g = divmod(c, 2)
        s = 384 * g
        # pair-major packing: [j] -> (q_j 128 cols | k_j 128 cols)
        wqk = np.empty((768, 3, 256), dtype=np.float32)
        bqk = np.empty((3, 2, 128), dtype=np.float32)
        for j in range(3):
            wqk[:, j, 0:128] = W[:, s + 128 * j: s + 128 * (j + 1)]
            wqk[:, j, 128:256] = W[:, 768 + s + 128 * j: 768 + s + 128 * (j + 1)]
            bqk[j, 0] = b[s + 128 * j: s + 128 * (j + 1)]
            bqk[j, 1] = b[768 + s + 128 * j: 768 + s + 128 * (j + 1)]
        in_maps.append({
            "xt": np.ascontiguousarray(x[bb].T.astype(BF16NP)),
            "wqk": np.ascontiguousarray(
                wqk.reshape(768, 768).astype(BF16NP)),
            "bqk": np.ascontiguousarray(bqk.reshape(768)),
            "wv": np.ascontiguousarray(W[:, 1536 + s:1536 + s + 384]
                                       .astype(BF16NP)),
            "wp": np.ascontiguousarray(Wp[s:s + 384, :].astype(BF16NP)),
        })
    return in_maps


def kernel(**inputs):
    x = np.asarray(inputs["x"], dtype=np.float32)
    b = np.asarray(inputs["b_attn"], dtype=np.float32)
    Wp = np.asarray(inputs["W_proj"], dtype=np.float32)
    bp = np.asarray(inputs["b_proj"], dtype=np.float32)
    B, T, C = x.shape

    if "nc" not in _CACHE:
        _CACHE["nc"] = build_nc(C, T, 6)
    nc = _CACHE["nc"]

    in_maps = make_in_maps(inputs)
    res = _run(nc, in_maps).results
    extra = (bp + b[1536:2304] @ Wp).astype(np.float32)  # [C]
    out = np.empty((B, T, C), dtype=np.float32)
    for bb in range(B):
        out[bb] = res[2 * bb]["out"] + res[2 * bb + 1]["out"] + extra
    return out


# revision 61
# speedup vs baseline: 20.0905x; 20.0905x over previous
"""Causal self-attention (B=4, T=2048, C=768, H=12, RoPE) on 8 TRN2 NeuronCores.

Sharding: core c -> (batch b = c//2, head-group g = c%2 of 6 heads).
Each core computes, for its batch element and its 6 heads:
    qkv projections, RoPE, causal attention, and the partial output
    projection  attn_out @ W_proj[rows of its heads].
Host sums the two partial outputs per batch and adds the (host-folded)
bias terms:  out[b] = part[2b] + part[2b+1] + b_proj + b_v @ W_proj.

v3 design (software-pipelined, bf16, DMA-lean):
  - All matmul operands bf16 (inputs pre-cast on host); psum f32.
  - Few, large input DMAs (HWDGE descriptor-gen serializes ~630ns/DMA).
  - RoPE row-swap done with a PE permutation matmul (no SBUF-SBUF DMAs).
  - Emission interleaving: pair-0 QK/V chunks are staged right before the
    query block that needs them; QK of pair j+1 and the output projection
    run as credit-scheduled "filler" PE work inside the ACT(exp)-bound
    attention loop, so no engine idles between phases.
  - Exact-causal trimming: the 4 diagonal key-chunks of each 512-query
    block run at widths 512/384/256/128; only the boundary [128,128]
    triangle gets the affine_select mask.
  - Scores computed transposed: S^T[k, q] = K^T-chunk @ Q-chunk, exp on
    ACT (scale=1/8), then Y'^T = [V|1]^T-chain accumulation giving Y^T
    (rows 0-63) and softmax denominators (row 64).  Normalization via
    DVE reciprocal + DRAM-broadcast multiply.
"""
import sys
sys.path.insert(0, "/opt/trn_rl_repo")

import numpy as np
import ml_dtypes

ROPE_BASE = 10000.0
NCORES = 8
BF16NP = ml_dtypes.bfloat16
FP8NP = ml_dtypes.float8_e4m3

_CACHE = {}


def _rope_tables(T):
    inv_freq = 1.0 / (ROPE_BASE ** (np.arange(0, 64, 2, dtype=np.float64) / 64))
    t = np.arange(T, dtype=np.float64)
    fr = np.outer(t, inv_freq)            # [T, 32]
    cosT = np.cos(fr).T                   # [32, T]
    sinT = np.sin(fr).T
    CC = np.tile(cosT, (4, 1)).astype(BF16NP)                      # [128, T]
    SS = np.concatenate([sinT, -sinT, sinT, -sinT], 0).astype(BF16NP)
    return CC, SS


def _swap_perm():
    """[128,128] P with P[swap(p), p] = 1 so (P.T @ x)[p] = x[swap(p)],
    swap = exchange rows 0-31<->32-63 and 64-95<->96-127."""
    P = np.zeros((128, 128), dtype=BF16NP)
    for p in range(128):
        h, r = divmod(p, 64)
        P[h * 64 + (r + 32) % 64, p] = 1.0
    return P


def build_nc(C, T, HPC, loop_n=1):
    """Per-core Bass program. C: contraction dim, T: seq len, HPC: heads."""
    import concourse.bass as bass
    import concourse.tile as tile
    from concourse import bacc, mybir
    import contextlib
    from collections import deque

    F32 = mybir.dt.float32
    BF16 = mybir.dt.bfloat16
    FP8 = mybir.dt.float8e4
    DR = mybir.MatmulPerfMode.DoubleRow
    Act = mybir.ActivationFunctionType

    KT = C // 128          # contraction k-tiles (6)
    NP = HPC // 2          # head pairs (3)
    TT = T // 128          # 128-row t-tiles / key chunks (16)
    QC = T // 512          # query blocks of 512 (4)
    VC = 64 * HPC          # v columns per core (384)
    PC = VC // 128         # projection contraction k-tiles (3)

    nc = bacc.Bacc("TRN2", target_bir_lowering=False, debug=False)

    # wqk is host-packed pair-major: [C, NP, 256] = [.., j, (q_j|k_j)]
    xt_d = nc.dram_tensor("xt", [C, T], BF16, kind="ExternalInput")
    wqk_d = nc.dram_tensor("wqk", [C, 2 * VC], BF16, kind="ExternalInput")
    bqk_d = nc.dram_tensor("bqk", [2 * VC], F32, kind="ExternalInput")
    wv_d = nc.dram_tensor("wv", [C, VC], BF16, kind="ExternalInput")
    wp_d = nc.dram_tensor("wp", [VC, C], BF16, kind="ExternalInput")
    out_d = nc.dram_tensor("out", [T, C], F32, kind="ExternalOutput")

    CCh, SSh = _rope_tables(T)
    cc_d = nc.inline_tensor(CCh, name="rope_cc")
    ss_d = nc.inline_tensor(SSh, name="rope_ss")
    psw_d = nc.inline_tensor(_swap_perm(), name="rope_swap")

    @contextlib.contextmanager
    def _maybe_loop(tc):
        if loop_n > 1:
            with tc.For_i(0, loop_n, 1):
                yield
        else:
            yield

    with nc.allow_low_precision(reason="bf16 matmul pipeline"), \
         tile.TileContext(nc) as tc:
        with contextlib.ExitStack() as octx, _maybe_loop(tc), \
             contextlib.ExitStack() as ctx:
            P = lambda name, bufs=1: ctx.enter_context(
                tc.tile_pool(name=name, bufs=bufs))
            sb = P("sb")
            xt_sb = sb.tile([128, KT, T], BF16, name="xt_sb")
            yt_sb = sb.tile([128, NP, T], BF16, name="yt_sb")
            wqk_sb = sb.tile([128, KT, NP, 256], BF16, name="wqk_sb")
            wv_sb = sb.tile([128, KT, VC], BF16, name="wv_sb")
            wp_sb = sb.tile([128, PC, C], BF16, name="wp_sb")
            # wp rows of the last pair's head-b, relocated to partitions 0-63
            wpb_sb = sb.tile([64, C], BF16, name="wpb_sb")
            # last pair's head-b Y rows (kept in SBUF; no partition-move DMA)
            tbs_sb = sb.tile([64, QC, 512], BF16, name="tbs_sb")
            # 72-col pad keeps the k-tile (DoubleRow Ko) byte-stride 16-aligned
            vp_sb = sb.tile([128, TT, HPC, 72], BF16, name="vp_sb")
            cc_sb = sb.tile([128, T], BF16, name="cc_sb")
            ss_sb = sb.tile([128, T], BF16, name="ss_sb")
            psw_sb = sb.tile([128, 128], BF16, name="psw_sb")
            bqk_sb = sb.tile([128, 2 * VC // 128], F32, name="bqk_sb")

            qk_pool = P("qks", bufs=2)
            pt_pool = P("pt", bufs=3)
            raw_pool = P("raw", bufs=2)
            t1_pool = P("t1", bufs=2)
            u_pool = P("u", bufs=2)
            yraw_pool = P("yraw", bufs=2)
            rd_pool = P("rd", bufs=2)
            tb_pool = P("tb", bufs=2)
            osb_pool = P("osb", bufs=3)

            mm_ps = ctx.enter_context(
                tc.tile_pool(name="mmps", bufs=2, space="PSUM"))
            qk_ps = ctx.enter_context(
                tc.tile_pool(name="qkps", bufs=2, space="PSUM"))
            y_ps = ctx.enter_context(
                tc.tile_pool(name="yps", bufs=1, space="PSUM"))

            # ---------------- loads (few, large, priority-ordered) -----
            nc.scalar.dma_start(
                xt_sb[:, :, 0:512],
                xt_d.ap()[:, 0:512].rearrange("(k p) t -> p k t", p=128))
            nc.sync.dma_start(
                wqk_sb[:, :, 0, :],
                wqk_d.ap()[:, 0:256].rearrange("(k p) c -> p k c", p=128))
            nc.sync.dma_start(
                bqk_sb[:], bqk_d.ap().rearrange("(m p) -> p m", p=128))
            nc.sync.dma_start(psw_sb, psw_d.ap())
            nc.scalar.dma_start(cc_sb[:, 0:1024], cc_d.ap()[:, 0:1024])
            nc.scalar.dma_start(ss_sb[:, 0:1024], ss_d.ap()[:, 0:1024])
            nc.scalar.dma_start(cc_sb[:, 1024:2048], cc_d.ap()[:, 1024:2048])
            nc.scalar.dma_start(ss_sb[:, 1024:2048], ss_d.ap()[:, 1024:2048])
            nc.sync.dma_start(
                wv_sb[:],
                wv_d.ap().rearrange("(k p) v -> p k v", p=128))
            nc.scalar.dma_start(
                xt_sb[:, :, 512:1024],
                xt_d.ap()[:, 512:1024].rearrange("(k p) t -> p k t", p=128))
            nc.scalar.dma_start(
                xt_sb[:, :, 1024:2048],
                xt_d.ap()[:, 1024:2048].rearrange("(k p) t -> p k t", p=128))
            for j2 in (1, 2):
                nc.sync.dma_start(
                    wqk_sb[:, :, j2, :],
                    wqk_d.ap()[:, 256 * j2:256 * (j2 + 1)].rearrange(
                        "(k p) c -> p k c", p=128))
            nc.sync.dma_start(
                wp_sb[:],
                wp_d.ap().rearrange("(k p) c -> p k c", p=128))
            nc.sync.dma_start(wpb_sb, wp_d.ap()[VC - 64:VC, :])
            nc.vector.tensor_copy(
                vp_sb[:, :, :, 64:65],
                nc.const_aps.tensor(1.0, (128, TT, HPC, 1)))
            ones_sb = sb.tile([65, 64], BF16, name="ones_sb")
            nc.vector.memset(ones_sb[64:65, :], 1.0)
            # exp bias of -2 keeps exp(s/8 - 2) inside fp8e4's +-240 range
            nb2_sb = sb.tile([128, 1], F32, name="nb2_sb")
            nc.vector.memset(nb2_sb, -2.0)

            # ---------------- emission helpers -------------------------
            pair_tiles = {}

            def get_pair(j):
                if j not in pair_tiles:
                    qt = qk_pool.tile([128, T], BF16, tag="qt", name=f"qt{j}")
                    kt = qk_pool.tile([128, T], BF16, tag="kt", name=f"kt{j}")
                    pair_tiles[j] = (qt, kt)
                return pair_tiles[j]

            def emit_qk_a(j, qk, ch):
                """QK chunk phase A: fp8 DoubleRow projection + bias -> raw."""
                cs = slice(ch * 512, (ch + 1) * 512)
                psum = qk_ps.tile([128, 512], F32, tag="qk", name="qkpsum")
                for k in range(KT):
                    nc.tensor.matmul(
                        psum, wqk_sb[:, k, j, qk * 128:(qk + 1) * 128],
                        xt_sb[:, k, cs], start=(k == 0), stop=(k == KT - 1))
                raw = raw_pool.tile([128, 512], BF16, name="raw")
                nc.vector.tensor_scalar_add(raw, psum, bqk_sb[:, 2 * j + qk:
                                                              2 * j + qk + 1])
                return psum, raw

            def emit_qk_b(j, qk, ch, psum, raw):
                """QK chunk phase B: rowswap via PE perm + RoPE combine.

                dst = raw*CC + rowswap(raw)*SS; the swap matmul overwrites
                the phase-A psum slot (its matmuls are already consumed).
                """
                dst = get_pair(j)[qk]
                cs = slice(ch * 512, (ch + 1) * 512)
                nc.tensor.matmul(psum, psw_sb, raw, start=True, stop=True)
                t1 = t1_pool.tile([128, 512], BF16, name="t1")
                nc.gpsimd.tensor_mul(t1, raw, cc_sb[:, cs])
                u = u_pool.tile([128, 512], BF16, name="u")
                nc.vector.tensor_mul(u, psum, ss_sb[:, cs])
                nc.vector.tensor_add(dst[:, cs], t1, u)

            def emit_qk_pair_chunks(j, ch):
                """Both m-tiles (q,k) of column chunk ch, phase-interleaved."""
                a0 = emit_qk_a(j, 0, ch)
                a1 = emit_qk_a(j, 1, ch)
                emit_qk_b(j, 0, ch, *a0)
                emit_qk_b(j, 1, ch, *a1)

            def emit_vp(tt):
                """V projection for key-chunk tt -> vp_sb[:, tt] (fp8 DR)."""
                psum = qk_ps.tile([128, 512], F32, tag="qk", name="vpsum")
                for k in range(KT):
                    nc.tensor.matmul(
                        psum[:, 0:VC], xt_sb[:, k, tt * 128:(tt + 1) * 128],
                        wv_sb[:, k, :], start=(k == 0), stop=(k == KT - 1))
                nc.vector.tensor_copy(
                    vp_sb[:, tt, :, 0:64],
                    psum[:, 0:VC].rearrange("p (h d) -> p h d", h=HPC))

            def emit_att_front(j, qc, kp, qt, kt):
                """Scores -> exp(fp8) -> causal mask for key-chunk pair
                (2kp, 2kp+1).  Both chunks of a diagonal pair run at the
                wider chunk's width; the extra strip is zeroed by the mask.
                Returns what the back half (PV) needs."""
                kc0 = 2 * kp
                po = max(0, (kc0 - 4 * qc) * 128)
                qs = slice(qc * 512 + po, (qc + 1) * 512)
                pp = pt_pool.tile([128, 2048], BF16, tag="pp", name="pp")
                ppv = pp.rearrange("p (r h q) -> p r h q", r=2, h=2)
                for r, kc in ((0, kc0), (1, kc0 + 1)):
                    ks = slice(kc * 128, (kc + 1) * 128)
                    eo = max(0, (kc - 4 * qc) * 128)  # exact chunk offset
                    eqs = slice(qc * 512 + eo, (qc + 1) * 512)
                    spair = mm_ps.tile([128, 1024], F32, tag="mm",
                                       name="spair")
                    nc.tensor.matmul(spair[:, eo:512], kt[0:64, ks],
                                     qt[0:64, eqs], start=True, stop=True)
                    nc.tensor.matmul(spair[:, 512 + eo:1024], kt[64:128, ks],
                                     qt[64:128, eqs], start=True, stop=True)
                    sview = spair.rearrange("p (h q) -> p h q",
                                            h=2)[:, :, eo:512]
                    nc.scalar.activation(ppv[:, r, :, eo:512], sview, Act.Exp,
                                         scale=0.125, bias=nb2_sb[:, 0:1])
                    if kc >= 4 * qc:  # mask k > q -> 0 in the triangle
                        tri = ppv[:, r, :, eo:eo + 128]
                        nc.gpsimd.affine_select(
                            out=tri, in_=tri,
                            compare_op=mybir.AluOpType.is_ge, fill=0.0,
                            base=0, channel_multiplier=-1,
                            pattern=[[0, 2], [1, 128]])
                return ppv, po

            def emit_att_back(j, qc, kp, ppv, po, yab, nkc):
                """PV accumulation for key-chunk pair kp (bf16, per chunk)."""
                kc0 = 2 * kp
                for r, kc in ((0, kc0), (1, kc0 + 1)):
                    eo = max(0, (kc - 4 * qc) * 128)
                    for h in (0, 1):
                        nc.tensor.matmul(
                            yab[:, 512 * h + eo:512 * (h + 1)],
                            vp_sb[:, kc, 2 * j + h, 0:65],
                            ppv[:, r, h, eo:512],
                            start=(kc == 0), stop=(kc == nkc - 1),
                            skip_group_check=True)

            def emit_norm_a(j, qc, yab):
                """Free the Y' psum: copy rows to SBUF, recip denominators."""
                rd = rd_pool.tile([65, 1024], BF16, name="rd")
                yr = yraw_pool.tile([64, 1024], BF16, name="yr")
                nc.vector.tensor_copy(yr, yab[0:64, :])
                nc.vector.reciprocal(rd[64:65, :], yab[64:65, :])
                return rd, yr

            def emit_norm_b(j, qc, rd, yr):
                """Broadcast 1/denom across partitions via a K=1 matmul,
                then scale Y^T rows into yt_sb (head-b of the last pair goes
                to tbs_sb; the projection consumes it directly)."""
                qs = slice(qc * 512, (qc + 1) * 512)
                bc0 = qk_ps.tile([128, 512], F32, tag="qk", name="bc0")
                nc.tensor.matmul(bc0[0:64, :], ones_sb[64:65, :],
                                 rd[64:65, 0:512], start=True, stop=True)
                bc1 = qk_ps.tile([128, 512], F32, tag="qk", name="bc1")
                nc.tensor.matmul(bc1[0:64, :], ones_sb[64:65, :],
                                 rd[64:65, 512:1024], start=True, stop=True)
                nc.vector.tensor_mul(yt_sb[0:64, j, qs], yr[:, 0:512],
                                     bc0[0:64, :])
                if j == NP - 1:
                    nc.vector.tensor_mul(tbs_sb[:, qc, :], yr[:, 512:1024],
                                         bc1[0:64, :])
                else:
                    tb = tb_pool.tile([64, 512], BF16, name="tb")
                    nc.vector.tensor_mul(tb, yr[:, 512:1024], bc1[0:64, :])
                    nc.sync.dma_start(yt_sb[64:128, j, qs], tb)

            tail_mode = [False]

            def emit_proj(tt, act_copy=False):
                act_copy = act_copy or tail_mode[0]
                ts = slice(tt * 128, (tt + 1) * 128)
                qc2, toff = divmod(tt, 4)
                osb = osb_pool.tile([128, C], F32, name="osb")
                for cc2 in range(2):
                    cs = slice(cc2 * 384, (cc2 + 1) * 384)
                    psum = qk_ps.tile([128, 512], F32, tag="qk", name="ppsum")
                    for k in range(PC - 1):
                        nc.tensor.matmul(
                            psum[:, 0:384], yt_sb[:, k, ts], wp_sb[:, k, cs],
                            start=(k == 0), stop=False)
                    # last pair: head-a from yt rows 0-63, head-b from tbs_sb
                    nc.tensor.matmul(
                        psum[:, 0:384], yt_sb[0:64, PC - 1, ts],
                        wp_sb[0:64, PC - 1, cs], start=False, stop=False,
                        skip_group_check=True)
                    nc.tensor.matmul(
                        psum[:, 0:384],
                        tbs_sb[:, qc2, toff * 128:(toff + 1) * 128],
                        wpb_sb[:, cs], start=False, stop=True,
                        skip_group_check=True)
                    if act_copy:
                        nc.scalar.copy(osb[:, cs], psum[:, 0:384])
                    else:
                        nc.vector.tensor_copy(osb[:, cs], psum[:, 0:384])
                nc.scalar.dma_start(out_d.ap()[ts, :], osb)

            # ---------------- pipelined emission ------------------------
            # Skewed software pipeline: the scores+exp "front" of step k+1
            # is emitted before the PV "back" of step k, so ACT (the binding
            # engine) is never starved by filler/normalize bursts.
            emit_qk_pair_chunks(0, 0)
            for tt in range(4):
                emit_vp(tt)

            fillers = deque()
            COST = {"qka": 500.0, "qkb": 300.0, "vp": 350.0, "proj": 1100.0}
            CREDIT = {0: 500.0, 1: 500.0, 2: 1100.0}

            def make_qk_fillers(jn, ch):
                state = {}
                def fa(q, c=ch, j_=jn):
                    state[q] = emit_qk_a(j_, q, c)
                def fb(q, c=ch, j_=jn):
                    emit_qk_b(j_, q, c, *state[q])
                return [("qka", (jn, ch), lambda: fa(0)),
                        ("qka", (jn, ch), lambda: fa(1)),
                        ("qkb", (jn, ch), lambda: fb(0)),
                        ("qkb", (jn, ch), lambda: fb(1))]

            def pump(credit_box, bump, cap=2600.0):
                credit_box[0] += bump
                popped = 0.0
                while (fillers and credit_box[0] >= COST[fillers[0][0]]
                       and popped < cap):
                    kind, key, fn = fillers.popleft()
                    credit_box[0] -= COST[kind]
                    popped += COST[kind]
                    fn()

            def drain_for(jt, qc):
                """Emit queued work att(jt, qc) depends on: pair-jt QK
                chunks <= qc and (for pair 0) V tiles <= 4qc+3."""
                while fillers:
                    kind, key, fn = fillers[0]
                    if kind in ("qka", "qkb") and key[0] == jt \
                            and key[1] <= qc:
                        pass
                    elif kind == "vp" and jt == 0 and key[1] <= 4 * qc + 3:
                        pass
                    else:
                        break
                    fillers.popleft()
                    fn()

            for ch in range(1, QC):
                fillers.extend(make_qk_fillers(0, ch))
                for tt in range(4 * ch, 4 * ch + 4):
                    fillers.append(("vp", ("v", tt),
                                    (lambda t=tt: emit_vp(t))))
            for ch in range(QC):
                fillers.extend(make_qk_fillers(1, ch))

            def emit_norm_b_and_projs(pn):
                pj, pqc = pn[0], pn[1]
                emit_norm_b(*pn)
                if pj == 2:
                    for tt in range(4 * pqc, 4 * pqc + 4):
                        fillers.append(
                            ("proj", None, (lambda t=tt: emit_proj(t))))

            steps = [(j, qc, kp) for j in range(NP) for qc in range(QC)
                     for kp in range(2 * (qc + 1))]
            credit_box = [0.0]
            prev = None
            pending_norm = None
            yab = None
            for (j, qc, kp) in steps:
                if kp == 0:
                    drain_for(j, qc)
                    if j == 1 and qc == 0:
                        for ch in range(QC):
                            fillers.extend(make_qk_fillers(2, ch))
                qt, kt = get_pair(j)
                fr = emit_att_front(j, qc, kp, qt, kt)
                if pending_norm is not None:
                    pump(credit_box, bump=900.0)
                    emit_norm_b_and_projs(pending_norm)
                    pending_norm = None
                if prev is not None:
                    (pj, pqc, pkp, pppv, ppo, pyab, pnkc) = prev
                    emit_att_back(pj, pqc, pkp, pppv, ppo, pyab, pnkc)
                    if pkp == 2 * (pqc + 1) - 1:
                        st = emit_norm_a(pj, pqc, pyab)
                        pending_norm = (pj, pqc) + st
                nkc = 4 * (qc + 1)
                if kp == 0:
                    yab = y_ps.tile([65, 1024], F32, tag="yab", name="yab")
                prev = (j, qc, kp, fr[0], fr[1], yab, nkc)
                pump(credit_box, bump=CREDIT[j])
            (pj, pqc, pkp, pppv, ppo, pyab, pnkc) = prev
            emit_att_back(pj, pqc, pkp, pppv, ppo, pyab, pnkc)
            st = emit_norm_a(pj, pqc, pyab)
            pump(credit_box, bump=2500.0)
            emit_norm_b_and_projs((pj, pqc) + st)
            tail_mode[0] = True
            while fillers:
                kind, key, fn = fillers.popleft()
                fn()
            for tt in range(4 * (QC - 1), TT):
                emit_proj(tt, act_copy=True)

    nc.compile()
    return nc


class _Runner:
    """Cached-jit SPMD runner (mirrors bass2jax.run_bass_via_pjrt, reusable)."""

    def __init__(self, nc, n_cores):
        import jax
        from jax.sharding import Mesh, PartitionSpec
        from jax.experimental.shard_map import shard_map
        import concourse.mybir as mybir
        from concourse import bass2jax

        bass2jax.install_neuronx_cc_hook()
        self.n_cores = n_cores
        part_name = (nc.partition_id_tensor.name
                     if nc.partition_id_tensor is not None else None)
        in_names, out_names, out_avals, zero_outs = [], [], [], []
        for alloc in nc.m.functions[0].allocations:
            if not isinstance(alloc, mybir.MemoryLocationSet):
                continue
            name = alloc.memorylocations[0].name
            if alloc.kind == "ExternalInput":
                if name != part_name:
                    in_names.append(name)
            elif alloc.kind == "ExternalOutput":
                out_names.append(name)
                shape = tuple(alloc.tensor_shape)
                dtype = mybir.dt.np(alloc.dtype)
                out_avals.append(jax.core.ShapedArray(shape, dtype))
                zero_outs.append(np.zeros(shape, dtype))
        self.in_names, self.out_names = in_names, out_names
        self.out_avals, self.zero_outs = out_avals, zero_outs
        all_names = in_names + out_names
        if part_name is not None:
            all_names = all_names + [part_name]

        def _body(*args):
            operands = list(args)
            if part_name is not None:
                operands.append(bass2jax.partition_id_tensor())
            return tuple(bass2jax._bass_exec_p.bind(
                *operands,
                out_avals=tuple(out_avals),
                in_names=tuple(all_names),
                out_names=tuple(out_names),
                lowering_input_output_aliases=(),
                sim_require_finite=True,
                sim_require_nnan=True,
                nc=nc,
            ))

        devices = jax.devices()[:n_cores]
        mesh = Mesh(np.asarray(devices), ("core",))
        nin = len(in_names) + len(out_names)
        self._fn = jax.jit(
            shard_map(_body, mesh=mesh,
                      in_specs=(PartitionSpec("core"),) * nin,
                      out_specs=(PartitionSpec("core"),) * len(out_names),
                      check_rep=False),
            keep_unused=True)

    def run(self, in_maps):
        args = [np.concatenate([np.asarray(m[name]) for m in in_maps], axis=0)
                for name in self.in_names]
        args += [np.zeros((self.n_cores * z.shape[0], *z.shape[1:]), z.dtype)
                 for z in self.zero_outs]
        outs = self._fn(*args)
        res = []
        for c in range(self.n_cores):
            d = {}
            for i, name in enumerate(self.out_names):
                per = np.asarray(outs[i]).reshape(
                    self.n_cores, *self.out_avals[i].shape)
                d[name] = per[c]
            res.append(d)
        return res


def _run(nc, in_maps):
    key = ("runner", id(nc))
    if key not in _CACHE:
        _CACHE[key] = _Runner(nc, len(in_maps))
    import types
    return types.SimpleNamespace(results=_CACHE[key].run(in_maps))


def make_in_maps(inputs):
    x = np.ascontiguousarray(np.asarray(inputs["x"], dtype=np.float32))
    W = np.asarray(inputs["W_attn"], dtype=np.float32)
    b = np.asarray(inputs["b_attn"], dtype=np.float32)
    Wp = np.asarray(inputs["W_proj"], dtype=np.float32)
    in_maps = []
    for c in range(NCORES):
        bb, g = divmod(c, 2)
        s = 384 * g
        # pair-major packing: [j] -> (q_j 128 cols | k_j 128 cols)
        wqk = np.empty((768, 3, 256), dtype=np.float32)
        bqk = np.empty((3, 2, 128), dtype=np.float32)
        for j in range(3):
            wqk[:, j, 0:128] = W[:, s + 128 * j: s + 128 * (j + 1)]
            wqk[:, j, 128:256] = W[:, 768 + s + 128 * j: 768 + s + 128 * (j + 1)]
            bqk[j, 0] = b[s + 128 * j: s + 128 * (j + 1)]
            bqk[j, 1] = b[768 + s + 128 * j: 768 + s + 128 * (j + 1)]
        in_maps.append({
            "xt": np.ascontiguousarray(x[bb].T.astype(BF16NP)),
            "wqk": np.ascontiguousarray(
                wqk.reshape(768, 768).astype(BF16NP)),
            "bqk": np.ascontiguousarray(bqk.reshape(768)),
            "wv": np.ascontiguousarray(W[:, 1536 + s:1536 + s + 384]
                                       .astype(BF16NP)),
            "wp": np.ascontiguousarray(Wp[s:s + 384, :].astype(BF16NP)),
        })
    return in_maps


def kernel(**inputs):
    x = np.asarray(inputs["x"], dtype=np.float32)
    b = np.asarray(inputs["b_attn"], dtype=np.float32)
    Wp = np.asarray(inputs["W_proj"], dtype=np.float32)
    bp = np.asarray(inputs["b_proj"], dtype=np.float32)
    B, T, C = x.shape

    if "nc" not in _CACHE:
        _CACHE["nc"] = build_nc(C, T, 6)
    nc = _CACHE["nc"]

    in_maps = make_in_maps(inputs)
    res = _run(nc, in_maps).results
    extra = (bp + b[1536:2304] @ Wp).astype(np.float32)  # [C]
    out = np.empty((B, T, C), dtype=np.float32)
    for bb in range(B):
        out[bb] = res[2 * bb]["out"] + res[2 * bb + 1]["out"] + extra
    return out


# revision 62
# speedup vs baseline: 20.1095x; 1.0009x over previous
"""Causal self-attention (B=4, T=2048, C=768, H=12, RoPE) on 8 TRN2 NeuronCores.

Sharding: core c -> (batch b = c//2, head-group g = c%2 of 6 heads).
Each core computes, for its batch element and its 6 heads:
    qkv projections, RoPE, causal attention, and the partial output
    projection  attn_out @ W_proj[rows of its heads].
Host sums the two partial outputs per batch and adds the (host-folded)
bias terms:  out[b] = part[2b] + part[2b+1] + b_proj + b_v @ W_proj.

v3 design (software-pipelined, bf16, DMA-lean):
  - All matmul operands bf16 (inputs pre-cast on host); psum f32.
  - Few, large input DMAs (HWDGE descriptor-gen serializes ~630ns/DMA).
  - RoPE row-swap done with a PE permutation matmul (no SBUF-SBUF DMAs).
  - Emission interleaving: pair-0 QK/V chunks are staged right before the
    query block that needs them; QK of pair j+1 and the output projection
    run as credit-scheduled "filler" PE work inside the ACT(exp)-bound
    attention loop, so no engine idles between phases.
  - Exact-causal trimming: the 4 diagonal key-chunks of each 512-query
    block run at widths 512/384/256/128; only the boundary [128,128]
    triangle gets the affine_select mask.
  - Scores computed transposed: S^T[k, q] = K^T-chunk @ Q-chunk, exp on
    ACT (scale=1/8), then Y'^T = [V|1]^T-chain accumulation giving Y^T
    (rows 0-63) and softmax denominators (row 64).  Normalization via
    DVE reciprocal + DRAM-broadcast multiply.
"""
import sys
sys.path.insert(0, "/opt/trn_rl_repo")

import numpy as np
import ml_dtypes

ROPE_BASE = 10000.0
NCORES = 8
BF16NP = ml_dtypes.bfloat16
FP8NP = ml_dtypes.float8_e4m3

_CACHE = {}


def _rope_tables(T):
    inv_freq = 1.0 / (ROPE_BASE ** (np.arange(0, 64, 2, dtype=np.float64) / 64))
    t = np.arange(T, dtype=np.float64)
    fr = np.outer(t, inv_freq)            # [T, 32]
    cosT = np.cos(fr).T                   # [32, T]
    sinT = np.sin(fr).T
    CC = np.tile(cosT, (4, 1)).astype(BF16NP)                      # [128, T]
    SS = np.concatenate([sinT, -sinT, sinT, -sinT], 0).astype(BF16NP)
    return CC, SS


def _swap_perm():
    """[128,128] P with P[swap(p), p] = 1 so (P.T @ x)[p] = x[swap(p)],
    swap = exchange rows 0-31<->32-63 and 64-95<->96-127."""
    P = np.zeros((128, 128), dtype=BF16NP)
    for p in range(128):
        h, r = divmod(p, 64)
        P[h * 64 + (r + 32) % 64, p] = 1.0
    return P


def build_nc(C, T, HPC, loop_n=1):
    """Per-core Bass program. C: contraction dim, T: seq len, HPC: heads."""
    import concourse.bass as bass
    import concourse.tile as tile
    from concourse import bacc, mybir
    import contextlib
    from collections import deque

    F32 = mybir.dt.float32
    BF16 = mybir.dt.bfloat16
    FP8 = mybir.dt.float8e4
    DR = mybir.MatmulPerfMode.DoubleRow
    Act = mybir.ActivationFunctionType

    KT = C // 128          # contraction k-tiles (6)
    NP = HPC // 2          # head pairs (3)
    TT = T // 128          # 128-row t-tiles / key chunks (16)
    QC = T // 512          # query blocks of 512 (4)
    VC = 64 * HPC          # v columns per core (384)
    PC = VC // 128         # projection contraction k-tiles (3)

    nc = bacc.Bacc("TRN2", target_bir_lowering=False, debug=False)

    # wqk is host-packed pair-major: [C, NP, 256] = [.., j, (q_j|k_j)]
    xt_d = nc.dram_tensor("xt", [C, T], BF16, kind="ExternalInput")
    wqk_d = nc.dram_tensor("wqk", [C, 2 * VC], BF16, kind="ExternalInput")
    bqk_d = nc.dram_tensor("bqk", [2 * VC], F32, kind="ExternalInput")
    wv_d = nc.dram_tensor("wv", [C, VC], BF16, kind="ExternalInput")
    wp_d = nc.dram_tensor("wp", [VC, C], BF16, kind="ExternalInput")
    out_d = nc.dram_tensor("out", [T, C], F32, kind="ExternalOutput")

    CCh, SSh = _rope_tables(T)
    cc_d = nc.inline_tensor(CCh, name="rope_cc")
    ss_d = nc.inline_tensor(SSh, name="rope_ss")
    psw_d = nc.inline_tensor(_swap_perm(), name="rope_swap")

    @contextlib.contextmanager
    def _maybe_loop(tc):
        if loop_n > 1:
            with tc.For_i(0, loop_n, 1):
                yield
        else:
            yield

    with nc.allow_low_precision(reason="bf16 matmul pipeline"), \
         tile.TileContext(nc) as tc:
        with contextlib.ExitStack() as octx, _maybe_loop(tc), \
             contextlib.ExitStack() as ctx:
            P = lambda name, bufs=1: ctx.enter_context(
                tc.tile_pool(name=name, bufs=bufs))
            sb = P("sb")
            xt_sb = sb.tile([128, KT, T], BF16, name="xt_sb")
            yt_sb = sb.tile([128, NP, T], BF16, name="yt_sb")
            wqk_sb = sb.tile([128, KT, NP, 256], BF16, name="wqk_sb")
            wv_sb = sb.tile([128, KT, VC], BF16, name="wv_sb")
            wp_sb = sb.tile([128, PC, C], BF16, name="wp_sb")
            # wp rows of the last pair's head-b, relocated to partitions 0-63
            wpb_sb = sb.tile([64, C], BF16, name="wpb_sb")
            # last pair's head-b Y rows (kept in SBUF; no partition-move DMA)
            tbs_sb = sb.tile([64, QC, 512], BF16, name="tbs_sb")
            # 72-col pad keeps the k-tile (DoubleRow Ko) byte-stride 16-aligned
            vp_sb = sb.tile([128, TT, HPC, 72], BF16, name="vp_sb")
            cc_sb = sb.tile([128, T], BF16, name="cc_sb")
            ss_sb = sb.tile([128, T], BF16, name="ss_sb")
            psw_sb = sb.tile([128, 128], BF16, name="psw_sb")
            bqk_sb = sb.tile([128, 2 * VC // 128], F32, name="bqk_sb")

            qk_pool = P("qks", bufs=2)
            pt_pool = P("pt", bufs=3)
            raw_pool = P("raw", bufs=2)
            t1_pool = P("t1", bufs=2)
            u_pool = P("u", bufs=2)
            yraw_pool = P("yraw", bufs=2)
            rd_pool = P("rd", bufs=2)
            tb_pool = P("tb", bufs=2)
            osb_pool = P("osb", bufs=3)

            mm_ps = ctx.enter_context(
                tc.tile_pool(name="mmps", bufs=2, space="PSUM"))
            qk_ps = ctx.enter_context(
                tc.tile_pool(name="qkps", bufs=2, space="PSUM"))
            y_ps = ctx.enter_context(
                tc.tile_pool(name="yps", bufs=1, space="PSUM"))

            # ---------------- loads (few, large, priority-ordered) -----
            nc.scalar.dma_start(
                xt_sb[:, :, 0:512],
                xt_d.ap()[:, 0:512].rearrange("(k p) t -> p k t", p=128))
            nc.sync.dma_start(
                wqk_sb[:, :, 0, :],
                wqk_d.ap()[:, 0:256].rearrange("(k p) c -> p k c", p=128))
            nc.sync.dma_start(
                bqk_sb[:], bqk_d.ap().rearrange("(m p) -> p m", p=128))
            nc.sync.dma_start(psw_sb, psw_d.ap())
            nc.sync.dma_start(
                wv_sb[:],
                wv_d.ap().rearrange("(k p) v -> p k v", p=128))
            nc.scalar.dma_start(cc_sb[:, 0:1024], cc_d.ap()[:, 0:1024])
            nc.scalar.dma_start(ss_sb[:, 0:1024], ss_d.ap()[:, 0:1024])
            nc.scalar.dma_start(cc_sb[:, 1024:2048], cc_d.ap()[:, 1024:2048])
            nc.scalar.dma_start(ss_sb[:, 1024:2048], ss_d.ap()[:, 1024:2048])
            nc.scalar.dma_start(
                xt_sb[:, :, 512:1024],
                xt_d.ap()[:, 512:1024].rearrange("(k p) t -> p k t", p=128))
            nc.scalar.dma_start(
                xt_sb[:, :, 1024:2048],
                xt_d.ap()[:, 1024:2048].rearrange("(k p) t -> p k t", p=128))
            for j2 in (1, 2):
                nc.sync.dma_start(
                    wqk_sb[:, :, j2, :],
                    wqk_d.ap()[:, 256 * j2:256 * (j2 + 1)].rearrange(
                        "(k p) c -> p k c", p=128))
            nc.sync.dma_start(
                wp_sb[:],
                wp_d.ap().rearrange("(k p) c -> p k c", p=128))
            nc.sync.dma_start(wpb_sb, wp_d.ap()[VC - 64:VC, :])
            nc.vector.tensor_copy(
                vp_sb[:, :, :, 64:65],
                nc.const_aps.tensor(1.0, (128, TT, HPC, 1)))
            ones_sb = sb.tile([65, 64], BF16, name="ones_sb")
            nc.vector.memset(ones_sb[64:65, :], 1.0)
            # exp bias of -2 keeps exp(s/8 - 2) inside fp8e4's +-240 range
            nb2_sb = sb.tile([128, 1], F32, name="nb2_sb")
            nc.vector.memset(nb2_sb, -2.0)

            # ---------------- emission helpers -------------------------
            pair_tiles = {}

            def get_pair(j):
                if j not in pair_tiles:
                    qt = qk_pool.tile([128, T], BF16, tag="qt", name=f"qt{j}")
                    kt = qk_pool.tile([128, T], BF16, tag="kt", name=f"kt{j}")
                    pair_tiles[j] = (qt, kt)
                return pair_tiles[j]

            def emit_qk_a(j, qk, ch):
                """QK chunk phase A: fp8 DoubleRow projection + bias -> raw."""
                cs = slice(ch * 512, (ch + 1) * 512)
                psum = qk_ps.tile([128, 512], F32, tag="qk", name="qkpsum")
                for k in range(KT):
                    nc.tensor.matmul(
                        psum, wqk_sb[:, k, j, qk * 128:(qk + 1) * 128],
                        xt_sb[:, k, cs], start=(k == 0), stop=(k == KT - 1))
                raw = raw_pool.tile([128, 512], BF16, name="raw")
                nc.vector.tensor_scalar_add(raw, psum, bqk_sb[:, 2 * j + qk:
                                                              2 * j + qk + 1])
                return psum, raw

            def emit_qk_b(j, qk, ch, psum, raw):
                """QK chunk phase B: rowswap via PE perm + RoPE combine.

                dst = raw*CC + rowswap(raw)*SS; the swap matmul overwrites
                the phase-A psum slot (its matmuls are already consumed).
                """
                dst = get_pair(j)[qk]
                cs = slice(ch * 512, (ch + 1) * 512)
                nc.tensor.matmul(psum, psw_sb, raw, start=True, stop=True)
                t1 = t1_pool.tile([128, 512], BF16, name="t1")
                nc.gpsimd.tensor_mul(t1, raw, cc_sb[:, cs])
                u = u_pool.tile([128, 512], BF16, name="u")
                nc.vector.tensor_mul(u, psum, ss_sb[:, cs])
                nc.vector.tensor_add(dst[:, cs], t1, u)

            def emit_qk_pair_chunks(j, ch):
                """Both m-tiles (q,k) of column chunk ch, phase-interleaved."""
                a0 = emit_qk_a(j, 0, ch)
                a1 = emit_qk_a(j, 1, ch)
                emit_qk_b(j, 0, ch, *a0)
                emit_qk_b(j, 1, ch, *a1)

            def emit_vp(tt):
                """V projection for key-chunk tt -> vp_sb[:, tt] (fp8 DR)."""
                psum = qk_ps.tile([128, 512], F32, tag="qk", name="vpsum")
                for k in range(KT):
                    nc.tensor.matmul(
                        psum[:, 0:VC], xt_sb[:, k, tt * 128:(tt + 1) * 128],
                        wv_sb[:, k, :], start=(k == 0), stop=(k == KT - 1))
                nc.vector.tensor_copy(
                    vp_sb[:, tt, :, 0:64],
                    psum[:, 0:VC].rearrange("p (h d) -> p h d", h=HPC))

            def emit_att_front(j, qc, kp, qt, kt):
                """Scores -> exp(fp8) -> causal mask for key-chunk pair
                (2kp, 2kp+1).  Both chunks of a diagonal pair run at the
                wider chunk's width; the extra strip is zeroed by the mask.
                Returns what the back half (PV) needs."""
                kc0 = 2 * kp
                po = max(0, (kc0 - 4 * qc) * 128)
                qs = slice(qc * 512 + po, (qc + 1) * 512)
                pp = pt_pool.tile([128, 2048], BF16, tag="pp", name="pp")
                ppv = pp.rearrange("p (r h q) -> p r h q", r=2, h=2)
                for r, kc in ((0, kc0), (1, kc0 + 1)):
                    ks = slice(kc * 128, (kc + 1) * 128)
                    eo = max(0, (kc - 4 * qc) * 128)  # exact chunk offset
                    eqs = slice(qc * 512 + eo, (qc + 1) * 512)
                    spair = mm_ps.tile([128, 1024], F32, tag="mm",
                                       name="spair")
                    nc.tensor.matmul(spair[:, eo:512], kt[0:64, ks],
                                     qt[0:64, eqs], start=True, stop=True)
                    nc.tensor.matmul(spair[:, 512 + eo:1024], kt[64:128, ks],
                                     qt[64:128, eqs], start=True, stop=True)
                    sview = spair.rearrange("p (h q) -> p h q",
                                            h=2)[:, :, eo:512]
                    nc.scalar.activation(ppv[:, r, :, eo:512], sview, Act.Exp,
                                         scale=0.125, bias=nb2_sb[:, 0:1])
                    if kc >= 4 * qc:  # mask k > q -> 0 in the triangle
                        tri = ppv[:, r, :, eo:eo + 128]
                        nc.gpsimd.affine_select(
                            out=tri, in_=tri,
                            compare_op=mybir.AluOpType.is_ge, fill=0.0,
                            base=0, channel_multiplier=-1,
                            pattern=[[0, 2], [1, 128]])
                return ppv, po

            def emit_att_back(j, qc, kp, ppv, po, yab, nkc):
                """PV accumulation for key-chunk pair kp (bf16, per chunk)."""
                kc0 = 2 * kp
                for r, kc in ((0, kc0), (1, kc0 + 1)):
                    eo = max(0, (kc - 4 * qc) * 128)
                    for h in (0, 1):
                        nc.tensor.matmul(
                            yab[:, 512 * h + eo:512 * (h + 1)],
                            vp_sb[:, kc, 2 * j + h, 0:65],
                            ppv[:, r, h, eo:512],
                            start=(kc == 0), stop=(kc == nkc - 1),
                            skip_group_check=True)

            def emit_norm_a(j, qc, yab):
                """Free the Y' psum: copy rows to SBUF, recip denominators."""
                rd = rd_pool.tile([65, 1024], BF16, name="rd")
                yr = yraw_pool.tile([64, 1024], BF16, name="yr")
                nc.vector.tensor_copy(yr, yab[0:64, :])
                nc.vector.reciprocal(rd[64:65, :], yab[64:65, :])
                return rd, yr

            def emit_norm_b(j, qc, rd, yr):
                """Broadcast 1/denom across partitions via a K=1 matmul,
                then scale Y^T rows into yt_sb (head-b of the last pair goes
                to tbs_sb; the projection consumes it directly)."""
                qs = slice(qc * 512, (qc + 1) * 512)
                bc0 = qk_ps.tile([128, 512], F32, tag="qk", name="bc0")
                nc.tensor.matmul(bc0[0:64, :], ones_sb[64:65, :],
                                 rd[64:65, 0:512], start=True, stop=True)
                bc1 = qk_ps.tile([128, 512], F32, tag="qk", name="bc1")
                nc.tensor.matmul(bc1[0:64, :], ones_sb[64:65, :],
                                 rd[64:65, 512:1024], start=True, stop=True)
                nc.vector.tensor_mul(yt_sb[0:64, j, qs], yr[:, 0:512],
                                     bc0[0:64, :])
                if j == NP - 1:
                    nc.vector.tensor_mul(tbs_sb[:, qc, :], yr[:, 512:1024],
                                         bc1[0:64, :])
                else:
                    tb = tb_pool.tile([64, 512], BF16, name="tb")
                    nc.vector.tensor_mul(tb, yr[:, 512:1024], bc1[0:64, :])
                    nc.sync.dma_start(yt_sb[64:128, j, qs], tb)

            tail_mode = [False]

            def emit_proj(tt, act_copy=False):
                act_copy = act_copy or tail_mode[0]
                ts = slice(tt * 128, (tt + 1) * 128)
                qc2, toff = divmod(tt, 4)
                osb = osb_pool.tile([128, C], F32, name="osb")
                for cc2 in range(2):
                    cs = slice(cc2 * 384, (cc2 + 1) * 384)
                    psum = qk_ps.tile([128, 512], F32, tag="qk", name="ppsum")
                    for k in range(PC - 1):
                        nc.tensor.matmul(
                            psum[:, 0:384], yt_sb[:, k, ts], wp_sb[:, k, cs],
                            start=(k == 0), stop=False)
                    # last pair: head-a from yt rows 0-63, head-b from tbs_sb
                    nc.tensor.matmul(
                        psum[:, 0:384], yt_sb[0:64, PC - 1, ts],
                        wp_sb[0:64, PC - 1, cs], start=False, stop=False,
                        skip_group_check=True)
                    nc.tensor.matmul(
                        psum[:, 0:384],
                        tbs_sb[:, qc2, toff * 128:(toff + 1) * 128],
                        wpb_sb[:, cs], start=False, stop=True,
                        skip_group_check=True)
                    if act_copy:
                        nc.scalar.copy(osb[:, cs], psum[:, 0:384])
                    else:
                        nc.vector.tensor_copy(osb[:, cs], psum[:, 0:384])
                nc.scalar.dma_start(out_d.ap()[ts, :], osb)

            # ---------------- pipelined emission ------------------------
            # Skewed software pipeline: the scores+exp "front" of step k+1
            # is emitted before the PV "back" of step k, so ACT (the binding
            # engine) is never starved by filler/normalize bursts.
            emit_qk_pair_chunks(0, 0)

            fillers = deque()
            COST = {"qka": 500.0, "qkb": 300.0, "vp": 350.0, "proj": 1100.0}
            CREDIT = {0: 500.0, 1: 500.0, 2: 1100.0}

            def make_qk_fillers(jn, ch):
                state = {}
                def fa(q, c=ch, j_=jn):
                    state[q] = emit_qk_a(j_, q, c)
                def fb(q, c=ch, j_=jn):
                    emit_qk_b(j_, q, c, *state[q])
                return [("qka", (jn, ch), lambda: fa(0)),
                        ("qka", (jn, ch), lambda: fa(1)),
                        ("qkb", (jn, ch), lambda: fb(0)),
                        ("qkb", (jn, ch), lambda: fb(1))]

            def pump(credit_box, bump, cap=2600.0):
                credit_box[0] += bump
                popped = 0.0
                while (fillers and credit_box[0] >= COST[fillers[0][0]]
                       and popped < cap):
                    kind, key, fn = fillers.popleft()
                    credit_box[0] -= COST[kind]
                    popped += COST[kind]
                    fn()

            def drain_for(jt, qc):
                """Emit queued work att(jt, qc) depends on: pair-jt QK
                chunks <= qc and (for pair 0) V tiles <= 4qc+3."""
                while fillers:
                    kind, key, fn = fillers[0]
                    if kind in ("qka", "qkb") and key[0] == jt \
                            and key[1] <= qc:
                        pass
                    elif kind == "vp" and jt == 0 and key[1] <= 4 * qc + 3:
                        pass
                    else:
                        break
                    fillers.popleft()
                    fn()

            for ch in range(1, QC):
                fillers.extend(make_qk_fillers(0, ch))
                for tt in range(4 * ch, 4 * ch + 4):
                    fillers.append(("vp", ("v", tt),
                                    (lambda t=tt: emit_vp(t))))
            for ch in range(QC):
                fillers.extend(make_qk_fillers(1, ch))

            def emit_norm_b_and_projs(pn):
                pj, pqc = pn[0], pn[1]
                emit_norm_b(*pn)
                if pj == 2:
                    for tt in range(4 * pqc, 4 * pqc + 4):
                        fillers.append(
                            ("proj", None, (lambda t=tt: emit_proj(t))))

            steps = [(j, qc, kp) for j in range(NP) for qc in range(QC)
                     for kp in range(2 * (qc + 1))]
            credit_box = [0.0]
            prev = None
            pending_norm = None
            yab = None
            for (j, qc, kp) in steps:
                if kp == 0:
                    drain_for(j, qc)
                    if j == 1 and qc == 0:
                        for ch in range(QC):
                            fillers.extend(make_qk_fillers(2, ch))
                qt, kt = get_pair(j)
                fr = emit_att_front(j, qc, kp, qt, kt)
                if (j, qc, kp) == (0, 0, 0):
                    for tt in range(4):
                        emit_vp(tt)
                if pending_norm is not None:
                    pump(credit_box, bump=900.0)
                    emit_norm_b_and_projs(pending_norm)
                    pending_norm = None
                if prev is not None:
                    (pj, pqc, pkp, pppv, ppo, pyab, pnkc) = prev
                    emit_att_back(pj, pqc, pkp, pppv, ppo, pyab, pnkc)
                    if pkp == 2 * (pqc + 1) - 1:
                        st = emit_norm_a(pj, pqc, pyab)
                        pending_norm = (pj, pqc) + st
                nkc = 4 * (qc + 1)
                if kp == 0:
                    yab = y_ps.tile([65, 1024], F32, tag="yab", name="yab")
                prev = (j, qc, kp, fr[0], fr[1], yab, nkc)
                pump(credit_box, bump=CREDIT[j])
            (pj, pqc, pkp, pppv, ppo, pyab, pnkc) = prev
            emit_att_back(pj, pqc, pkp, pppv, ppo, pyab, pnkc)
            st = emit_norm_a(pj, pqc, pyab)
            pump(credit_box, bump=2500.0)
            emit_norm_b_and_projs((pj, pqc) + st)
            tail_mode[0] = True
            while fillers:
                kind, key, fn = fillers.popleft()
                fn()
            for tt in range(4 * (QC - 1), TT):
                emit_proj(tt, act_copy=True)

    nc.compile()
    return nc


class _Runner:
    """Cached-jit SPMD runner (mirrors bass2jax.run_bass_via_pjrt, reusable)."""

    def __init__(self, nc, n_cores):
        import jax
        from jax.sharding import Mesh, PartitionSpec
        from jax.experimental.shard_map import shard_map
        import concourse.mybir as mybir
        from concourse import bass2jax

        bass2jax.install_neuronx_cc_hook()
        self.n_cores = n_cores
        part_name = (nc.partition_id_tensor.name
                     if nc.partition_id_tensor is not None else None)
        in_names, out_names, out_avals, zero_outs = [], [], [], []
        for alloc in nc.m.functions[0].allocations:
            if not isinstance(alloc, mybir.MemoryLocationSet):
                continue
            name = alloc.memorylocations[0].name
            if alloc.kind == "ExternalInput":
                if name != part_name:
                    in_names.append(name)
            elif alloc.kind == "ExternalOutput":
                out_names.append(name)
                shape = tuple(alloc.tensor_shape)
                dtype = mybir.dt.np(alloc.dtype)
                out_avals.append(jax.core.ShapedArray(shape, dtype))
                zero_outs.append(np.zeros(shape, dtype))
        self.in_names, self.out_names = in_names, out_names
        self.out_avals, self.zero_outs = out_avals, zero_outs
        all_names = in_names + out_names
        if part_name is not None:
            all_names = all_names + [part_name]

        def _body(*args):
            operands = list(args)
            if part_name is not None:
                operands.append(bass2jax.partition_id_tensor())
            return tuple(bass2jax._bass_exec_p.bind(
                *operands,
                out_avals=tuple(out_avals),
                in_names=tuple(all_names),
                out_names=tuple(out_names),
                lowering_input_output_aliases=(),
                sim_require_finite=True,
                sim_require_nnan=True,
                nc=nc,
            ))

        devices = jax.devices()[:n_cores]
        mesh = Mesh(np.asarray(devices), ("core",))
        nin = len(in_names) + len(out_names)
        self._fn = jax.jit(
            shard_map(_body, mesh=mesh,
                      in_specs=(PartitionSpec("core"),) * nin,
                      out_specs=(PartitionSpec("core"),) * len(out_names),
                      check_rep=False),
            keep_unused=True)

    def run(self, in_maps):
        args = [np.concatenate([np.asarray(m[name]) for m in in_maps], axis=0)
                for name in self.in_names]
        args += [np.zeros((self.n_cores * z.shape[0], *z.shape[1:]), z.dtype)
                 for z in self.zero_outs]
        outs = self._fn(*args)
        res = []
        for c in range(self.n_cores):
            d = {}
            for i, name in enumerate(self.out_names):
                per = np.asarray(outs[i]).reshape(
                    self.n_cores, *self.out_avals[i].shape)
                d[name] = per[c]
            res.append(d)
        return res


def _run(nc, in_maps):
    key = ("runner", id(nc))
    if key not in _CACHE:
        _CACHE[key] = _Runner(nc, len(in_maps))
    import types
    return types.SimpleNamespace(results=_CACHE[key].run(in_maps))


def make_in_maps(inputs):
    x = np.ascontiguousarray(np.asarray(inputs["x"], dtype=np.float32))
    W = np.asarray(inputs["W_attn"], dtype=np.float32)
    b = np.asarray(inputs["b_attn"], dtype=np.float32)
    Wp = np.asarray(inputs["W_proj"], dtype=np.float32)
    in_maps = []
    for c in range(NCORES):
        bb, g = divmod(c, 2)
        s = 384 * g
        # pair-major packing: [j] -> (q_j 128 cols | k_j 128 cols)
        wqk = np.empty((768, 3, 256), dtype=np.float32)
        bqk = np.empty((3, 2, 128), dtype=np.float32)
        for j in range(3):
            wqk[:, j, 0:128] = W[:, s + 128 * j: s + 128 * (j + 1)]
            wqk[:, j, 128:256] = W[:, 768 + s + 128 * j: 768 + s + 128 * (j + 1)]
            bqk[j, 0] = b[s + 128 * j: s + 128 * (j + 1)]
            bqk[j, 1] = b[768 + s + 128 * j: 768 + s + 128 * (j + 1)]
        in_maps.append({
            "xt": np.ascontiguousarray(x[bb].T.astype(BF16NP)),
            "wqk": np.ascontiguousarray(
                wqk.reshape(768, 768).astype(BF16NP)),
            "bqk": np.ascontiguousarray(bqk.reshape(768)),
            "wv": np.ascontiguousarray(W[:, 1536 + s:1536 + s + 384]
                                       .astype(BF16NP)),
            "wp": np.ascontiguousarray(Wp[s:s + 384, :].astype(BF16NP)),
        })
    return in_maps


def kernel(**inputs):
    x = np.asarray(inputs["x"], dtype=np.float32)
    b = np.asarray(inputs["b_attn"], dtype=np.float32)
    Wp = np.asarray(inputs["W_proj"], dtype=np.float32)
    bp = np.asarray(inputs["b_proj"], dtype=np.float32)
    B, T, C = x.shape

    if "nc" not in _CACHE:
        _CACHE["nc"] = build_nc(C, T, 6)
    nc = _CACHE["nc"]

    in_maps = make_in_maps(inputs)
    res = _run(nc, in_maps).results
    extra = (bp + b[1536:2304] @ Wp).astype(np.float32)  # [C]
    out = np.empty((B, T, C), dtype=np.float32)
    for bb in range(B):
        out[bb] = res[2 * bb]["out"] + res[2 * bb + 1]["out"] + extra
    return out
